# revision 47
# baseline (speedup 1.0000x reference)
"""Trainium2 Bass kernel for 3-layer GAT (nn_MultiLayerGAT).

Strategy (dst-node sharding, 8 cores): bf16 datapath, gather-minimized.
  - Nodes are assigned to 80 blocks of 128 by a degree-balanced LPT
    permutation (minimizes CK, the padded chunk count that sets the gather
    descriptor budget; host un-permutes the output rows). Core k owns
    blocks [10k, 10k+10). Each block's edge list is padded to a uniform CK
    chunks of 128 edges (same CK for all cores => one SPMD program).
  - Per layer:
    Phase A (replicated): xe = h @ W_ext (bf16 matmul) -> HBM rows
        [xp(256) | al_s(8) | al_d(8) | pad] (384 bf16; layer3: 128 bf16:
        [xp(40) | al_s(1) | al_d(1) | pad])
      where W_ext = [W | W@a_src_blockdiag | W@a_dst_blockdiag].
    Phase B (sharded): per dst block:
      - dma_gather of xe rows by src (768B/edge bf16), split 4 ways
        across the 4 SWDGE queues with the sub-gather->ring mapping
        rotated per block (decorrelates head-of-line coupling between
        consecutive blocks' drains; ~80-100us/rep on HW). The gathers are
        drain-latency bound (~6 ns/descriptor aggregate).
        Slot d of chunk 0 holds dst d's self-loop, so the gathered chunk 0
        doubles as the block's own al_d table (no separate DMA, correct
        per-core under SPMD).
      - al_d per edge via PE matmul: ohT[d,e] (host-precomputed one-hot
        transpose, bf16, DMA-streamed per block; building it on-chip via
        PE transposes measured ~250us slower) times ad_loc[d,h] ->
        ald[e,h] in PSUM. No per-edge AD gather (halves descriptor count
        vs a per-edge al_d gather).
      - e = lrelu(al_s[src] + al_d[dst]); ee = exp(e) (no max subtraction
        -- validated safe); scale gathered features by ee; segment-sum via
        one-hot matmul on TensorE (bf16): ps[128 dst, 264] += oh^T @
        [ee*G | ee]. Divide by the summed ee column, bias, ELU, transpose
        (layers 1-2) for the next layer's lhsT.
    Phase C (layers 1-2): AllGather of transposed h shards (bf16) so every
      core has the full h^T for the next layer's Phase A.
  - Final layer: single head, out = log_softmax(out_pre/s + b3) per block.

Pads: gather idx 0 (finite garbage), dstcode -1 => one-hot column (both oh
and ohT) is all zero, so pads contribute nothing anywhere.
"""

import numpy as np

N = 10000
E = 320000
IN = 128
HID = 32
HEADS = 8
HC = HEADS * HID          # 256
OUT = 40
NEG = 0.2

NPAD = 10240              # 80 blocks of 128
NBLK_TOT = NPAD // 128    # 80
NCORES = 8
NB = NBLK_TOT // NCORES   # 10 blocks per core

ROW12 = 384               # xe row elems (bf16), layers 1-2
ROW3 = 128                # layer 3
ALD12 = 264               # al_d col offset, layers 1-2 (al_s at 256)
ALD3 = 41                 # layer 3 (al_s at 40)
NQ = 4                    # SWDGE queues (ucode max)
GSPLIT = 4                # sub-gathers per block, one per queue
POS = None                # node -> padded slot permutation (set by preprocess)


# ----------------------------------------------------------------------------
# host-side preprocessing
# ----------------------------------------------------------------------------

def build_w_ext(W, a_src, a_dst, row):
    """W_ext[in, row] bf16: [W | W@As | W@Ad | pad] with As/Ad block-diag."""
    inn, hc = W.shape
    H, C = a_src.shape
    As = np.zeros((hc, H), np.float32)
    Ad = np.zeros((hc, H), np.float32)
    for h in range(H):
        As[h * C:(h + 1) * C, h] = a_src[h]
        Ad[h * C:(h + 1) * C, h] = a_dst[h]
    We = np.zeros((inn, row), np.float32)
    We[:, 0:hc] = W
    We[:, hc:hc + H] = W @ As
    We[:, hc + H:hc + 2 * H] = W @ Ad
    return We


def preprocess(edge_index):
    """Chunk tables shared by all layers. Returns (CK, per-core arrays).

    Slot layout per block: the self-loop edge of dst-code d sits at slot d
    (chunk 0, partition d), so the gathered chunk 0 doubles as the block's
    own al_d table (G[:, 0, ALD:ALD+nh]) and chunk 0's one-hot is the
    identity. Remaining edges fill slots 128.. in dst-sorted order.
    """
    src0 = np.asarray(edge_index[0], np.int64)
    dst0 = np.asarray(edge_index[1], np.int64)

    # Degree-balanced node->slot permutation (LPT): assign nodes to the 80
    # blocks so every block carries ~E/80 edges; this minimizes CK (the
    # padded chunk count that sets the gather descriptor budget). pos[node]
    # is the padded slot id; the caller un-permutes rows on the host.
    deg = np.bincount(dst0, minlength=N)
    order_deg = np.argsort(-deg, kind="stable")
    load = np.zeros(NBLK_TOT, np.int64)
    room = np.full(NBLK_TOT, 128, np.int64)
    nblk = np.zeros(N, np.int64)
    for nd in order_deg:
        cands = np.where(room > 0)[0]
        b = cands[np.argmin(load[cands])]
        nblk[nd] = b
        load[b] += deg[nd]
        room[b] -= 1
    pos = np.zeros(N, np.int64)
    nxt = np.zeros(NBLK_TOT, np.int64)
    for nd in range(N):
        b = nblk[nd]
        pos[nd] = 128 * b + nxt[b]
        nxt[b] += 1

    src = pos[src0]
    dst = pos[dst0]
    order = np.argsort(dst, kind="stable")
    ssrc, sdst = src[order], dst[order]
    blk = sdst // 128                                  # block of each edge
    cnt = np.bincount(blk, minlength=NBLK_TOT)
    # +128 self-loop slots (chunk 0) per block; non-loop edges start at 128
    CK = int(np.ceil((cnt.max() + 128) / 128))
    S = CK * 128                                       # slots per block
    starts = np.concatenate([[0], np.cumsum(cnt)])

    global POS
    POS = pos

    gsrc = np.zeros((NBLK_TOT, S), np.int64)           # gather idx (by src)
    dstc = np.full((NBLK_TOT, S), -1, np.int64)        # dst - 128*block
    for b in range(NBLK_TOT):
        # chunk 0: self-loops of the real nodes in this block
        nreal = int(nxt[b])
        d = np.arange(nreal)
        gsrc[b, :nreal] = 128 * b + d
        dstc[b, :nreal] = d
        # remaining edges (the original E, no self-loops) from slot 128,
        # sorted by src (pads keep idx 0 so their gathered values stay
        # finite; negative skip-indices crash the gather ucode here)
        lo, hi = starts[b], starts[b + 1]
        n = hi - lo
        o = np.argsort(ssrc[lo:hi], kind="stable")
        gsrc[b, 128:128 + n] = ssrc[lo:hi][o]
        dstc[b, 128:128 + n] = sdst[lo:hi][o] - 128 * b

    def wrap16(idx_flat):
        # [S] -> [128, S//16] int16 tile; idx i -> [i%16, i//16], replicated
        # to all 128 partitions (each GPSIMD Q7 core reads its own group)
        t16 = idx_flat.reshape(S // 16, 16).T.astype(np.int16)
        return np.tile(t16, (8, 1))

    d128 = np.arange(128)
    cores = []
    for k in range(NCORES):
        bsl = slice(k * NB, (k + 1) * NB)
        gsrc_t = np.concatenate(
            [wrap16(gsrc[b]) for b in range(k * NB, (k + 1) * NB)], axis=1)
        # dstc tile: [128, NB*CK], col (b*CK+j)[p] = code of edge j*128+p
        dc = dstc[bsl].reshape(NB, CK, 128).transpose(2, 0, 1).reshape(128, NB * CK)
        # ohT: [128, NB*CK*128] bf16; ohT[d, (b*CK+j)*128+e] = 1 iff
        # dstcode(edge j*128+e of block b) == d
        codes = dstc[bsl].reshape(NB * CK, 128)        # [chunks, e]
        oht = (codes[None, :, :] == d128[:, None, None])
        oht = oht.reshape(128, NB * CK * 128)
        cores.append(dict(
            gsrc=gsrc_t,
            dstc=dc.astype(np.float32),  # cast to ml_bfloat16 at ship time
            oht=oht))
    return CK, cores


def _bf16(a):
    """Round f32 ndarray to bfloat16 via jax's numpy dtype."""
    import jax.numpy as jnp
    return np.asarray(jnp.asarray(a, jnp.bfloat16))


def build_in_maps(inputs, cores):
    x = np.asarray(inputs["x"], np.float32)
    xTn = np.zeros((IN, NPAD), np.float32)
    xTn[:, POS] = x.T
    W1en = build_w_ext(np.asarray(inputs["W1"], np.float32),
                       np.asarray(inputs["a_src1"], np.float32),
                       np.asarray(inputs["a_dst1"], np.float32), ROW12)
    W2en = build_w_ext(np.asarray(inputs["W2"], np.float32),
                       np.asarray(inputs["a_src2"], np.float32),
                       np.asarray(inputs["a_dst2"], np.float32), ROW12)
    W3en = build_w_ext(np.asarray(inputs["W3"], np.float32),
                       np.asarray(inputs["a_src3"], np.float32),
                       np.asarray(inputs["a_dst3"], np.float32), ROW3)
    iota_n = np.tile(np.arange(128, dtype=np.float32), (128, 1))
    ident_n = np.eye(128, dtype=np.float32)
    b1n = np.tile(np.asarray(inputs["b1"], np.float32), (128, 1))
    b2n = np.tile(np.asarray(inputs["b2"], np.float32), (128, 1))
    b3n = np.tile(np.asarray(inputs["b3"], np.float32), (128, 1))

    in_maps = []
    for k in range(NCORES):
        c = cores[k]
        in_maps.append({
            "xT": _bf16(xTn), "W1e": _bf16(W1en), "W2e": _bf16(W2en),
            "W3e": _bf16(W3en), "gsrc": c["gsrc"], "dstc": c["dstc"],
            "ohT": _bf16(c["oht"].astype(np.float32)),
            "iota": _bf16(iota_n), "ident": _bf16(ident_n),
            "b1r": b1n, "b2r": b2n, "b3r": b3n,
        })
    return in_maps


# ----------------------------------------------------------------------------
# bass program
# ----------------------------------------------------------------------------

def build_nc(CK):
    import os
    import concourse.bacc as bacc
    import concourse.mybir as mybir
    import concourse.tile as tile
    from concourse.library_config import mlp

    f32 = mybir.dt.float32
    bf16 = mybir.dt.bfloat16
    i16 = mybir.dt.int16
    Alu = mybir.AluOpType
    Act = mybir.ActivationFunctionType

    S = CK * 128

    GB = int(os.environ.get("GAT_GB", "3"))
    GS = int(os.environ.get("GAT_GS", str(GSPLIT)))
    QROT = bool(int(os.environ.get("GAT_QROT", "1")))
    LB = int(os.environ.get("GAT_LB", "3"))
    XB = int(os.environ.get("GAT_XB", "3"))
    OB = int(os.environ.get("GAT_OB", "2"))
    nc = bacc.Bacc("TRN2", debug=False, num_swdge_queues=NQ)

    # inputs (per core)
    xT = nc.dram_tensor("xT", [IN, NPAD], bf16, kind="ExternalInput")
    W1e = nc.dram_tensor("W1e", [IN, ROW12], bf16, kind="ExternalInput")
    W2e = nc.dram_tensor("W2e", [HC, ROW12], bf16, kind="ExternalInput")
    W3e = nc.dram_tensor("W3e", [HC, ROW3], bf16, kind="ExternalInput")
    gsrc = nc.dram_tensor("gsrc", [128, NB * S // 16], i16, kind="ExternalInput")
    dstc = nc.dram_tensor("dstc", [128, NB * CK], f32, kind="ExternalInput")
    ohT = nc.dram_tensor("ohT", [128, NB * CK * 128], bf16, kind="ExternalInput")
    iota = nc.dram_tensor("iota", [128, 128], bf16, kind="ExternalInput")
    ident = nc.dram_tensor("ident", [128, 128], bf16, kind="ExternalInput")
    b1r = nc.dram_tensor("b1r", [128, HC], f32, kind="ExternalInput")
    b2r = nc.dram_tensor("b2r", [128, HC], f32, kind="ExternalInput")
    b3r = nc.dram_tensor("b3r", [128, OUT], f32, kind="ExternalInput")

    out = nc.dram_tensor("out", [NB * 128, OUT], f32, kind="ExternalOutput")

    # scratch DRAM
    xe12a = nc.dram_tensor("xe12a", [NPAD, ROW12], bf16)
    xe12b = nc.dram_tensor("xe12b", [NPAD, ROW12], bf16)
    xe3 = nc.dram_tensor("xe3", [NPAD, ROW3], bf16)
    AGS = int(os.environ.get("GAT_AGS", "8"))  # blocks 0..AGS-1 AG'd early
    cc_in1a = nc.dram_tensor("cc_in1a", [HC, AGS * 128], bf16)
    cc_in1b = nc.dram_tensor("cc_in1b", [HC, (NB - AGS) * 128], bf16)
    cc_out1a = nc.dram_tensor("cc_out1a", [NCORES * HC, AGS * 128], bf16,
                              addr_space="Shared")
    cc_out1b = nc.dram_tensor("cc_out1b", [NCORES * HC, (NB - AGS) * 128],
                              bf16, addr_space="Shared")
    cc_in2a = nc.dram_tensor("cc_in2a", [HC, AGS * 128], bf16)
    cc_in2b = nc.dram_tensor("cc_in2b", [HC, (NB - AGS) * 128], bf16)
    cc_out2a = nc.dram_tensor("cc_out2a", [NCORES * HC, AGS * 128], bf16,
                              addr_space="Shared")
    cc_out2b = nc.dram_tensor("cc_out2b", [NCORES * HC, (NB - AGS) * 128],
                              bf16, addr_space="Shared")
    cc_min_in = nc.dram_tensor("cc_min_in", [128, 16], bf16)
    cc_min_out = nc.dram_tensor("cc_min_out", [NCORES * 128, 16], bf16,
                                addr_space="Shared")

    with tile.TileContext(nc) as tc:
        nc.gpsimd.load_library(mlp)
        with tc.tile_pool(name="const", bufs=1) as cpool, \
             tc.tile_pool(name="w", bufs=1) as wpool, \
             tc.tile_pool(name="lhs", bufs=LB) as lhspool, \
             tc.tile_pool(name="xps", bufs=XB) as xpspool, \
             tc.tile_pool(name="gath", bufs=GB) as gpool, \
             tc.tile_pool(name="oht", bufs=OB) as otpool, \
             tc.tile_pool(name="ohall", bufs=2) as oapool, \
             tc.tile_pool(name="small", bufs=2) as spool, \
             tc.tile_pool(name="oh", bufs=4) as ohpool, \
             tc.tile_pool(name="post", bufs=2) as ppool, \
             tc.tile_pool(name="psA", bufs=2, space="PSUM") as psA, \
             tc.tile_pool(name="psB", bufs=2, space="PSUM") as psB, \
             tc.tile_pool(name="psD", bufs=2, space="PSUM") as psD, \
             tc.tile_pool(name="psT", bufs=2, space="PSUM") as psT:

            # constants resident in SBUF
            gsrc_t = cpool.tile([128, NB * S // 16], i16, tag="gsrc")
            nc.sync.dma_start(gsrc_t[:], gsrc[:])
            dstc_t = cpool.tile([128, NB * CK], f32, tag="dstc")
            nc.sync.dma_start(dstc_t[:], dstc[:])
            iota_t = cpool.tile([128, 128], bf16, tag="iota")
            nc.sync.dma_start(iota_t[:], iota[:])
            ident_t = cpool.tile([128, 128], bf16, tag="ident")
            nc.sync.dma_start(ident_t[:], ident[:])
            b1_t = cpool.tile([128, HC], f32, tag="b1")
            nc.sync.dma_start(b1_t[:], b1r[:])
            b2_t = cpool.tile([128, HC], f32, tag="b2")
            nc.sync.dma_start(b2_t[:], b2r[:])
            b3_t = cpool.tile([128, OUT], f32, tag="b3")
            nc.sync.dma_start(b3_t[:], b3r[:])

            def phase_a(layer, we_dram, row, xe_dram, cc_halves):
                """xe = h @ W_ext for all NPAD nodes -> xe_dram (bf16).

                lhs tiles are loaded in NB-block batches (one DMA per
                source-core x kchunk) and xe written in NB-block batches.
                """
                kchunks = 1 if layer == 1 else 2
                w_t = wpool.tile([128, kchunks, row], bf16, tag=f"w{layer}")
                for kk in range(kchunks):
                    nc.sync.dma_start(w_t[:, kk, :], we_dram[kk * 128:(kk + 1) * 128, :])
                for src in range(NCORES):
                    lhs = lhspool.tile([128, kchunks, NB * 128], bf16, tag="lhs")
                    for kk in range(kchunks):
                        if layer == 1:
                            nc.sync.dma_start(
                                lhs[:, kk, :],
                                xT[:, src * NB * 128:(src + 1) * NB * 128])
                        else:
                            g = src * HC + kk * 128
                            for half, (cco, c0, c1) in enumerate(cc_halves):
                                nc.sync.dma_start(
                                    lhs[:, kk, c0 * 128:c1 * 128],
                                    cco[g:g + 128, 0:(c1 - c0) * 128])
                    sb = xpspool.tile([128, NB, row], bf16, tag="xpsb")
                    for c in range(NB):
                        ps = psA.tile([128, row], f32, tag="xps")
                        for kk in range(kchunks):
                            nc.tensor.matmul(
                                ps[:], lhs[:, kk, c * 128:(c + 1) * 128],
                                w_t[:, kk, :],
                                start=(kk == 0), stop=(kk == kchunks - 1))
                        nc.vector.tensor_copy(sb[:, c, :], ps[:])
                    t0 = src * NB
                    nc.sync.dma_start(
                        xe_dram[t0 * 128:(t0 + AGS) * 128, :]
                        .rearrange("(c p) r -> p c r", p=128), sb[:, 0:AGS, :])
                    nc.sync.dma_start(
                        xe_dram[(t0 + AGS) * 128:(t0 + NB) * 128, :]
                        .rearrange("(c p) r -> p c r", p=128), sb[:, AGS:NB, :])

            def phase_b(layer, row, xe_dram, cc_in_dram, b_t, agcb=None):
                """aggregation over this core's NB blocks. cc_in_dram is a
                (half_a, half_b) pair of DRAM tensors for the transposed h
                shard; agcb() is invoked after block AGS-1 (early AllGather
                of half a, overlapped with the remaining blocks)."""
                nh = HEADS if layer < 3 else 1
                fe = HC if layer < 3 else OUT          # feature width
                rw = fe + nh                           # matmul rhs width
                alo = fe                               # al_s offset in row
                ado = ALD12 if layer < 3 else ALD3     # al_d col in xe

                for b in range(NB):
                    oht_t = otpool.tile([128, CK, 128], bf16, tag="ohT")
                    nc.sync.dma_start(
                        oht_t[:].rearrange("p c e -> p (c e)"),
                        ohT[:, b * CK * 128:(b + 1) * CK * 128])

                    g_t = gpool.tile([128, CK, row], bf16, tag="G")
                    cpq = max(1, CK // GS)
                    for q in range(GS):
                        j0 = min(CK, q * cpq)
                        j1 = CK if q == GS - 1 else min(CK, (q + 1) * cpq)
                        n = (j1 - j0) * 128
                        if n == 0:
                            continue
                        isl = gsrc_t[:, b * S // 16 + j0 * 8:
                                     b * S // 16 + j1 * 8]
                        qn = (q + b) % NQ if QROT else q % NQ
                        nc.gpsimd.dma_gather(
                            g_t[:, j0:j1, :], xe_dram[:, 0:row], isl, n, n,
                            row, elem_step=row, single_packet=False,
                            queue_num=qn)

                    # own-block al_d table = chunk 0's gathered rows
                    # (self-loop of dst d sits at slot d)
                    ad_loc = g_t[:, 0, ado:ado + nh]
                    # al_d per edge via PE: ald[e, h] = sum_d ohT[d,e] ad[d,h]
                    ps_ald = psD.tile([128, CK, nh], f32, tag="ald")
                    for j in range(CK):
                        nc.tensor.matmul(
                            ps_ald[:, j, :], oht_t[:, j, :],
                            ad_loc, start=True, stop=True)

                    # e = lrelu(al_s + al_d); ee = exp(e)
                    ee_t = spool.tile([128, CK, nh], bf16, tag="ee")
                    nc.vector.tensor_tensor(
                        ee_t[:], g_t[:, :, alo:alo + nh], ps_ald[:], Alu.add)
                    eef = ee_t[:].rearrange("p c h -> p (c h)")
                    nc.vector.scalar_tensor_tensor(
                        eef, eef, NEG, eef, Alu.mult, Alu.max)
                    nc.scalar.activation(eef, eef, Act.Exp)

                    # scale features in place, stash ee next to them
                    if layer < 3:
                        nc.vector.tensor_tensor(
                            g_t[:, :, 0:fe].rearrange("p c (h z) -> p c h z", z=HID),
                            g_t[:, :, 0:fe].rearrange("p c (h z) -> p c h z", z=HID),
                            ee_t[:].to_broadcast([128, CK, nh, HID]),
                            Alu.mult)
                    else:
                        nc.vector.tensor_tensor(
                            g_t[:, :, 0:fe],
                            g_t[:, :, 0:fe],
                            ee_t[:].rearrange("p c h -> p (c h)").to_broadcast([128, CK, fe]),
                            Alu.mult)
                    nc.vector.tensor_copy(g_t[:, :, fe:fe + nh], ee_t[:])

                    # one-hot tiles and the segment matmul
                    ps = psB.tile([128, rw], f32, tag="agg")
                    for j in range(CK):
                        oh_t = ohpool.tile([128, 128], bf16, tag="oh")
                        nc.vector.tensor_scalar(
                            oh_t[:], iota_t[:],
                            dstc_t[:, b * CK + j:b * CK + j + 1], None,
                            Alu.is_equal)
                        nc.tensor.matmul(
                            ps[:], oh_t[:], g_t[:, j, 0:rw],
                            start=(j == 0), stop=(j == CK - 1))

                    # divide by ee-sum, bias
                    r_t = spool.tile([128, nh], f32, tag="recip")
                    nc.vector.tensor_scalar(
                        r_t[:], ps[:, fe:fe + nh], 1e-16, None, Alu.add)
                    nc.vector.reciprocal(r_t[:], r_t[:])
                    h_t = ppool.tile([128, fe], f32, tag="H")
                    if layer < 3:
                        nc.vector.tensor_tensor(
                            h_t[:].rearrange("p (h z) -> p h z", z=HID),
                            ps[:, 0:fe].rearrange("p (h z) -> p h z", z=HID),
                            r_t[:].to_broadcast([128, nh, HID]),
                            Alu.mult)
                    else:
                        nc.vector.tensor_scalar(
                            h_t[:], ps[:, 0:fe], r_t[:], None, Alu.mult)
                    nc.vector.tensor_tensor(h_t[:], h_t[:], b_t[:], Alu.add)

                    if layer < 3:
                        # ELU: relu(z) + exp(min(z,0)) - 1, then transpose
                        t2 = ppool.tile([128, fe], f32, tag="elu")
                        nc.vector.tensor_scalar(t2[:], h_t[:], 0.0, None, Alu.min)
                        nc.scalar.activation(t2[:], t2[:], Act.Exp)
                        nc.vector.scalar_tensor_tensor(
                            h_t[:], h_t[:], 0.0, t2[:], Alu.max, Alu.add)
                        hb = ppool.tile([128, fe], bf16, tag="hb")
                        nc.vector.tensor_scalar(hb[:], h_t[:], -1.0, None, Alu.add)
                        cdst, cb = ((cc_in_dram[0], b) if b < AGS
                                    else (cc_in_dram[1], b - AGS))
                        for half in range(2):
                            pt = psT.tile([128, 128], bf16, tag="tr")
                            nc.tensor.transpose(
                                pt[:], hb[:, half * 128:(half + 1) * 128],
                                ident_t[:])
                            st = ppool.tile([128, 128], bf16, tag="trs")
                            nc.vector.tensor_copy(st[:], pt[:])
                            nc.sync.dma_start(
                                cdst[half * 128:(half + 1) * 128,
                                     cb * 128:(cb + 1) * 128], st[:])
                        if b == AGS - 1 and agcb is not None:
                            agcb()
                    else:
                        # log_softmax over the 40 classes
                        m_t = spool.tile([128, 1], f32, tag="m")
                        nc.vector.tensor_reduce(
                            m_t[:], h_t[:], mybir.AxisListType.X, Alu.max)
                        nc.vector.tensor_scalar(
                            h_t[:], h_t[:], m_t[:], None, Alu.subtract)
                        x_t = ppool.tile([128, fe], f32, tag="exps")
                        s_t = spool.tile([128, 1], f32, tag="s")
                        nc.scalar.activation(
                            x_t[:], h_t[:], Act.Exp, accum_out=s_t[:])
                        l_t = spool.tile([128, 1], f32, tag="l")
                        nc.scalar.activation(l_t[:], s_t[:], Act.Ln)
                        nc.vector.tensor_scalar(
                            h_t[:], h_t[:], l_t[:], None, Alu.subtract)
                        nc.sync.dma_start(
                            out[b * 128:(b + 1) * 128, :], h_t[:])

            mode = os.environ.get("GAT_MODE", "full")
            reps = int(os.environ.get("GAT_REPS", "1"))
            nocc = (mode == "nocc")
            ccmin = (mode == "ccmin")

            def allgather(cin, cout):
                if nocc:
                    return
                if ccmin:
                    cin, cout = cc_min_in, cc_min_out
                nc.gpsimd.collective_compute(
                    "AllGather", mybir.AluOpType.bypass,
                    replica_groups=[list(range(NCORES))],
                    ins=[cin.ap().opt()], outs=[cout.ap().opt()])

            halves1 = [(cc_out1a, 0, AGS), (cc_out1b, AGS, NB)]
            halves2 = [(cc_out2a, 0, AGS), (cc_out2b, AGS, NB)]
            for _rep in range(reps):
                # layer 1
                phase_a(1, W1e, ROW12, xe12a, None)
                phase_b(1, ROW12, xe12a, (cc_in1a, cc_in1b), b1_t,
                        agcb=lambda: allgather(cc_in1a, cc_out1a))
                allgather(cc_in1b, cc_out1b)
                # layer 2
                phase_a(2, W2e, ROW12, xe12b, halves1)
                phase_b(2, ROW12, xe12b, (cc_in2a, cc_in2b), b2_t,
                        agcb=lambda: allgather(cc_in2a, cc_out2a))
                allgather(cc_in2b, cc_out2b)
                # layer 3
                phase_a(3, W3e, ROW3, xe3, halves2)
                phase_b(3, ROW3, xe3, None, b3_t)

    nc.compile()
    return nc


# ----------------------------------------------------------------------------
# entry point
# ----------------------------------------------------------------------------

LAST_EXEC_NS = None


def kernel(**inputs):
    import os
    from concourse.bass_utils import run_bass_kernel_spmd
    global LAST_EXEC_NS

    ei = np.asarray(inputs["edge_index"])
    CK, cores = preprocess(ei)
    in_maps = build_in_maps(inputs, cores)
    nc = build_nc(CK)
    trace = bool(int(os.environ.get("GAT_TRACE", "0")))
    res = run_bass_kernel_spmd(nc, in_maps, list(range(NCORES)), trace=trace)
    LAST_EXEC_NS = res.exec_time_ns
    full = np.concatenate([res.results[k]["out"] for k in range(NCORES)], axis=0)
    return full[POS].astype(np.float32)
